# revision 22
# baseline (speedup 1.0000x reference)
"""AQT int8-quantized matmul (dynamic symmetric quantization) on 8 TRN2 cores.

Full problem: lhs [8192, 4096] f32 @ rhs [4096, 4096] f32 with per-row lhs
scales and per-column rhs scales (abs-max / 127.5), int8 round+clip, int32
matmul, dequantize by the outer product of scales.

Sharding: 2x4 grid over (M, N). Each core gets lhs rows M/2 (natural layout)
and the TRANSPOSE of its rhs column block (rhsT [N_loc, K], built on host
during sharding), computes its [4096, 1024] output block; host assembles the
8 blocks. Both quantization axes keep their full contraction dim per core, so
per-core results match the unsharded reference. No collectives.

Per-core kernel (V5 — PE-bound design):
- rhs path: rhsT n-tiles [128, K] quantize with PER-PARTITION scales: DVE
  absmax reduce, ScalarE magic-round pass 1 (x*r + 1.5*2^23), then ONE DVE
  tensor_scalar (sub magic, mult s_r) that both finishes the round and FOLDS
  the per-column scale s_r into the quantized bf16 values (~1.7e-3 rel err,
  well inside tolerance). Tiles are DMA-xbar-transposed into two K-major
  resident buffers qr[nb] [128, KT, 512]. Load and quant stages are emitted
  with a one-tile skew: the engine queues are strict FIFO, so emission order
  decides head-of-line blocking on the DVE->ScalarE->DVE chain.
- lhs path per m-tile: natural-layout full-width quant (DVE absmax reduce +
  two ScalarE ACTs over [128, 4096]), then the bf16 tile is transposed to
  K-major. For the first HEAD_MIS m-tiles ALL k-tiles go through TensorE
  transpose-mode matmuls (the PE is idle during the rhs phase and this keeps
  the serial Sync/xbar queue free for the rhs transposes); later m-tiles
  split XBAR_KT k-tiles to the DMA xbar and the rest to the PE, keeping the
  DMA engines under ~80% so the steady state never stalls the PE.
- matmul: nb-outer accumulation groups (32 matmuls of [128x128]@[128x512]);
  the first HEAD_MIS m-tiles run all their nb0 groups before any nb1 group
  so the PE has work while n-tiles 4-7 still quantize. PSUM eviction applies
  the per-row scale s_l via ScalarE activation scale (s_r already folded).
  int8 values ride bf16 exactly; fp32 PSUM accumulation reproduces the int32
  matmul.
"""
import sys

if "/opt/trn_rl_repo" not in sys.path:
    sys.path.insert(0, "/opt/trn_rl_repo")

from contextlib import ExitStack

import numpy as np

from concourse import bacc, masks, mybir, tile
from concourse.bass_utils import run_bass_kernel_spmd

f32 = mybir.dt.float32
bf16 = mybir.dt.bfloat16
Alu = mybir.AluOpType
Act = mybir.ActivationFunctionType

P = 128
C_MAGIC = 1.5 * 2 ** 23
QDIV = 127.5 * (1.0 - 2.0 ** -20)
INV_QDIV = 1.0 / QDIV
TINY = 1e-30

M, K, N = 8192, 4096, 4096
MG, NG = 2, 4                      # shard grid rows (M) x cols (N)
M_loc, N_loc = M // MG, N // NG    # 4096, 1024 per core
N_CORES = MG * NG

XBAR_KT = 24                       # k-tiles per m-tile transposed via DMA xbar
HEAD_MIS = 3                       # m-tiles run nb0-first + all-PE transpose


def build_aqt(nc, M_loc, K, N_loc, W=512):
    KT, MT = K // P, M_loc // P          # 32, 32
    NB = N_loc // W                      # 2
    NT = N_loc // P                      # 8 rhs n-tiles
    TPB = NT // NB                       # n-tiles per nb block (4)
    H = K // 2                           # rhs half width (2048)
    HT = H // P                          # 16 k-tiles per half
    assert (KT - XBAR_KT) % 8 == 0

    lhs = nc.declare_dram_parameter("lhs", [M_loc, K], f32, isOutput=False)
    rhsT = nc.declare_dram_parameter("rhsT", [N_loc, K], f32, isOutput=False)
    out = nc.declare_dram_parameter("out", [M_loc, N_loc], f32, isOutput=True)

    with tile.TileContext(nc) as tc, ExitStack() as ctx:
        pool = lambda name, bufs: ctx.enter_context(tc.tile_pool(name=name, bufs=bufs))
        const_pool = pool("constp", 1)
        qr_pool = pool("qr", 1)            # quantized+scaled rhs, K-major, resident
        raws = pool("raws", 4)             # unified raw staging [P, K] f32
        rt1 = pool("rt1", 1)               # rhs scaled+C halves f32
        rqf = pool("rqf", 2)               # rhs quantized*s_r halves bf16
        rsc = pool("rsc", 4)               # rhs scale columns [P, 1]
        lt1 = pool("lt1", 1)               # lhs scaled+C [P, K] f32
        lqb = pool("lqb", 2)               # lhs quantized [P, K] bf16
        lqt = pool("lqt", 3)               # lhs quantized transposed [P, KT, P]
        lsc = pool("lsc", 1)               # s_l columns, resident
        sml = pool("sml", 6)               # [P, 1] scratch
        opool = pool("o1", 2)
        psum = ctx.enter_context(tc.tile_pool(name="psum", bufs=3, space="PSUM"))
        psumT = ctx.enter_context(tc.tile_pool(name="psumT", bufs=2, space="PSUM"))

        ident = const_pool.tile([P, P], bf16)
        masks.make_identity(nc, ident[:])

        s_l_all = lsc.tile([P, MT], f32)
        qr_nb = [qr_pool.tile([P, KT, W], bf16, name=f"qrnb{nb}")
                 for nb in range(NB)]

        # ---- rhs stages: load+reduce, then quant+fold+xbar ----
        rraw_t, rsc_t = {}, {}

        def emit_rhs_load(j):
            raw = raws.tile([P, K], f32, name="raw")
            nc.sync.dma_start(raw[:], rhsT[j * P:(j + 1) * P, :])
            am = sml.tile([P, 1], f32, name="ram")
            nc.vector.tensor_reduce(am[:], raw[:], axis=mybir.AxisListType.X,
                                    op=Alu.max, apply_absolute_value=True)
            s_col = rsc.tile([P, 1], f32, name="rs")
            nc.vector.tensor_scalar(s_col[:], am[:], TINY, INV_QDIV,
                                    op0=Alu.max, op1=Alu.mult)
            r_col = sml.tile([P, 1], f32, name="rr")
            nc.vector.reciprocal(r_col[:], s_col[:])
            rraw_t[j] = raw
            rsc_t[j] = (s_col, r_col)

        def emit_rhs_quant(j):
            nb, jo = divmod(j, TPB)
            raw = rraw_t.pop(j)
            s_col, r_col = rsc_t.pop(j)
            for h in range(2):
                t1 = rt1.tile([P, H], f32, name="rt1")
                nc.scalar.activation(t1[:], raw[:, h * H:(h + 1) * H], Act.Copy,
                                     bias=C_MAGIC, scale=r_col[:])
                qf = rqf.tile([P, H], bf16, name="rqf")
                nc.vector.tensor_scalar(qf[:], t1[:], C_MAGIC, s_col[:],
                                        op0=Alu.subtract, op1=Alu.mult)
                nc.sync.dma_start_transpose(
                    qr_nb[nb][:, h * HT:(h + 1) * HT, jo * P:(jo + 1) * P], qf[:])

        # ---- lhs pipeline stages (full-width [P, K] tiles) ----
        lraw_t, lam_t, lqb_t, lqt_t = {}, {}, {}, {}

        def emit_lhs_load(mi):
            rs = slice(mi * P, (mi + 1) * P)
            raw = raws.tile([P, K], f32, name="raw")
            nc.sync.dma_start(raw[:], lhs[rs, :])
            am = sml.tile([P, 1], f32, name="lam")
            nc.vector.tensor_reduce(am[:], raw[:], axis=mybir.AxisListType.X,
                                    op=Alu.max, apply_absolute_value=True)
            lraw_t[mi] = raw
            lam_t[mi] = am

        def emit_lhs_quant(mi):
            raw, am = lraw_t.pop(mi), lam_t.pop(mi)
            s_col = s_l_all[:, mi:mi + 1]
            nc.vector.tensor_scalar(s_col, am[:], TINY, INV_QDIV,
                                    op0=Alu.max, op1=Alu.mult)
            r_col = sml.tile([P, 1], f32, name="lr")
            nc.vector.reciprocal(r_col[:], s_col)
            t1 = lt1.tile([P, K], f32, name="lt1")
            nc.scalar.activation(t1[:], raw[:], Act.Copy,
                                 bias=C_MAGIC, scale=r_col[:])
            qb = lqb.tile([P, K], bf16, name="lqb")
            nc.scalar.activation(qb[:], t1[:], Act.Copy, bias=-C_MAGIC)
            lqb_t[mi] = qb

        def emit_lhs_transpose(mi, xbar_kt):
            qb = lqb_t.pop(mi)
            qt = lqt.tile([P, KT, P], bf16, name="lqt")
            if xbar_kt > 0:
                nc.sync.dma_start_transpose(qt[:, 0:xbar_kt, :],
                                            qb[:, 0:xbar_kt * P])
            for g in range((KT - xbar_kt) // 8):
                pt = psumT.tile([P, 8 * P], bf16, name="pt")
                for t in range(8):
                    kt = xbar_kt + g * 8 + t
                    nc.tensor.transpose(pt[:, t * P:(t + 1) * P],
                                        qb[:, kt * P:(kt + 1) * P],
                                        ident[:])
                nc.vector.tensor_copy(qt[:, xbar_kt + g * 8:xbar_kt + (g + 1) * 8, :],
                                      pt[:])
            lqt_t[mi] = qt

        def emit_mm_group(mi, nb, last):
            qt = lqt_t[mi]
            if last:
                del lqt_t[mi]
            rs = slice(mi * P, (mi + 1) * P)
            ps = psum.tile([P, W], f32, name="ps")
            for kt in range(KT):
                nc.tensor.matmul(ps[:], qt[:, kt, :], qr_nb[nb][:, kt, :],
                                 start=(kt == 0), stop=(kt == KT - 1))
            o1 = opool.tile([P, W], f32, name="o1")
            nc.scalar.activation(o1[:], ps[:], Act.Copy, bias=0.0,
                                 scale=s_l_all[:, mi:mi + 1])
            nc.sync.dma_start(out[rs, nb * W:(nb + 1) * W], o1[:])

        # ---- emission schedule: slot-audited for the shared 4-buf raw
        # pool; rhs tiles get slot priority so tile 7 lands early, lhs
        # loads 4/5 pre-issued so the pipeline does not restart after the
        # head, steady loop runs a 3-deep lhs prefetch ----
        emit_lhs_load(0)
        emit_rhs_load(0)
        emit_rhs_load(1)
        emit_rhs_load(2)
        emit_lhs_quant(0)
        emit_lhs_transpose(0, 0)
        emit_lhs_load(1)
        emit_rhs_quant(0)
        emit_rhs_load(3)
        emit_rhs_quant(1)
        emit_rhs_load(4)
        emit_rhs_quant(2)
        emit_rhs_load(5)
        emit_lhs_quant(1)
        emit_lhs_transpose(1, 0)
        emit_rhs_quant(3)
        emit_rhs_load(6)
        emit_rhs_quant(4)
        emit_rhs_load(7)
        emit_lhs_load(2)
        emit_lhs_quant(2)
        emit_lhs_transpose(2, 0)
        emit_rhs_quant(5)
        emit_lhs_load(3)
        emit_rhs_quant(6)
        emit_rhs_quant(7)
        emit_lhs_quant(3)
        emit_lhs_transpose(3, XBAR_KT)
        emit_lhs_load(4)
        emit_lhs_load(5)
        for mi in range(HEAD_MIS):
            emit_mm_group(mi, 0, last=False)
        for mi in range(HEAD_MIS):
            emit_mm_group(mi, 1, last=True)
        emit_lhs_quant(4)
        emit_lhs_transpose(4, XBAR_KT)
        emit_lhs_load(6)

        for mi in range(HEAD_MIS, MT):
            if mi >= HEAD_MIS + 1:
                if mi + 3 < MT:
                    emit_lhs_load(mi + 3)
                if mi + 1 < MT and mi + 1 > HEAD_MIS + 1:
                    emit_lhs_quant(mi + 1)
                    emit_lhs_transpose(mi + 1, XBAR_KT)
            for nb in range(NB):
                emit_mm_group(mi, nb, last=(nb == NB - 1))
    return nc


_COMPILED_NC = None


def _get_compiled():
    global _COMPILED_NC
    if _COMPILED_NC is None:
        nc = bacc.Bacc("TRN2", target_bir_lowering=False, debug=False,
                       num_devices=N_CORES)
        build_aqt(nc, M_loc, K, N_loc)
        nc.compile()
        _COMPILED_NC = nc
    return _COMPILED_NC


def _shard(lhs, rhs):
    rhsT = np.ascontiguousarray(rhs.T)   # [N, K]; row slices stay contiguous
    in_maps = []
    for i in range(N_CORES):
        mg, ng = divmod(i, NG)
        in_maps.append({
            "lhs": np.ascontiguousarray(lhs[mg * M_loc:(mg + 1) * M_loc, :]),
            "rhsT": rhsT[ng * N_loc:(ng + 1) * N_loc, :],
        })
    return in_maps


def kernel(lhs, rhs, _trace=False, _trace_kwargs=None):
    lhs = np.asarray(lhs, np.float32)
    rhs = np.asarray(rhs, np.float32)
    nc = _get_compiled()
    res = run_bass_kernel_spmd(nc, _shard(lhs, rhs), core_ids=list(range(N_CORES)),
                               trace=_trace, **(_trace_kwargs or {}))
    out = np.empty((M, N), np.float32)
    for i in range(N_CORES):
        mg, ng = divmod(i, NG)
        out[mg * M_loc:(mg + 1) * M_loc, ng * N_loc:(ng + 1) * N_loc] = \
            res.results[i]["out"]
    kernel.last_result = res
    return out


# revision 23
# speedup vs baseline: 1.0097x; 1.0097x over previous
"""AQT int8-quantized matmul (dynamic symmetric quantization) on 8 TRN2 cores.

Full problem: lhs [8192, 4096] f32 @ rhs [4096, 4096] f32 with per-row lhs
scales and per-column rhs scales (abs-max / 127.5), int8 round+clip, int32
matmul, dequantize by the outer product of scales.

Sharding: 2x4 grid over (M, N). Each core gets lhs rows M/2 (natural layout)
and the TRANSPOSE of its rhs column block (rhsT [N_loc, K], built on host
during sharding), computes its [4096, 1024] output block; host assembles the
8 blocks. Both quantization axes keep their full contraction dim per core, so
per-core results match the unsharded reference. No collectives.

Per-core kernel (V5 — PE-bound design):
- rhs path: rhsT n-tiles [128, K] quantize with PER-PARTITION scales: DVE
  absmax reduce, ScalarE magic-round pass 1 (x*r + 1.5*2^23), then ONE DVE
  tensor_scalar (sub magic, mult s_r) that both finishes the round and FOLDS
  the per-column scale s_r into the quantized bf16 values (~1.7e-3 rel err,
  well inside tolerance). Tiles are DMA-xbar-transposed into two K-major
  resident buffers qr[nb] [128, KT, 512]. Load and quant stages are emitted
  with a one-tile skew: the engine queues are strict FIFO, so emission order
  decides head-of-line blocking on the DVE->ScalarE->DVE chain.
- lhs path per m-tile: natural-layout full-width quant (DVE absmax reduce +
  two ScalarE ACTs over [128, 4096]), then the bf16 tile is transposed to
  K-major. For the first HEAD_MIS m-tiles ALL k-tiles go through TensorE
  transpose-mode matmuls (the PE is idle during the rhs phase and this keeps
  the serial Sync/xbar queue free for the rhs transposes); later m-tiles
  split XBAR_KT k-tiles to the DMA xbar and the rest to the PE, keeping the
  DMA engines under ~80% so the steady state never stalls the PE.
- matmul: nb-outer accumulation groups (32 matmuls of [128x128]@[128x512]);
  the first HEAD_MIS m-tiles run all their nb0 groups before any nb1 group
  so the PE has work while n-tiles 4-7 still quantize. PSUM eviction applies
  the per-row scale s_l via ScalarE activation scale (s_r already folded).
  int8 values ride bf16 exactly; fp32 PSUM accumulation reproduces the int32
  matmul.
"""
import sys

if "/opt/trn_rl_repo" not in sys.path:
    sys.path.insert(0, "/opt/trn_rl_repo")

from contextlib import ExitStack

import numpy as np

from concourse import bacc, masks, mybir, tile
from concourse.bass_utils import run_bass_kernel_spmd

f32 = mybir.dt.float32
bf16 = mybir.dt.bfloat16
Alu = mybir.AluOpType
Act = mybir.ActivationFunctionType

P = 128
C_MAGIC = 1.5 * 2 ** 23
QDIV = 127.5 * (1.0 - 2.0 ** -20)
INV_QDIV = 1.0 / QDIV
TINY = 1e-30

M, K, N = 8192, 4096, 4096
MG, NG = 2, 4                      # shard grid rows (M) x cols (N)
M_loc, N_loc = M // MG, N // NG    # 4096, 1024 per core
N_CORES = MG * NG

XBAR_KT = 24                       # k-tiles per m-tile transposed via DMA xbar
HEAD_MIS = 3                       # m-tiles run nb0-first + all-PE transpose


def build_aqt(nc, M_loc, K, N_loc, W=512):
    KT, MT = K // P, M_loc // P          # 32, 32
    NB = N_loc // W                      # 2
    NT = N_loc // P                      # 8 rhs n-tiles
    TPB = NT // NB                       # n-tiles per nb block (4)
    H = K // 2                           # rhs half width (2048)
    HT = H // P                          # 16 k-tiles per half
    assert (KT - XBAR_KT) % 8 == 0

    lhs = nc.declare_dram_parameter("lhs", [M_loc, K], f32, isOutput=False)
    rhsT = nc.declare_dram_parameter("rhsT", [N_loc, K], f32, isOutput=False)
    out = nc.declare_dram_parameter("out", [M_loc, N_loc], f32, isOutput=True)

    with tile.TileContext(nc) as tc, ExitStack() as ctx:
        pool = lambda name, bufs: ctx.enter_context(tc.tile_pool(name=name, bufs=bufs))
        const_pool = pool("constp", 1)
        qr_pool = pool("qr", 1)            # quantized+scaled rhs, K-major, resident
        raws = pool("raws", 4)             # unified raw staging [P, K] f32
        rt1 = pool("rt1", 1)               # rhs scaled+C halves f32
        rqf = pool("rqf", 2)               # rhs quantized*s_r halves bf16
        rsc = pool("rsc", 4)               # rhs scale columns [P, 1]
        lt1 = pool("lt1", 1)               # lhs scaled+C [P, K] f32
        lqb = pool("lqb", 2)               # lhs quantized [P, K] bf16
        lqt = pool("lqt", 3)               # lhs quantized transposed [P, KT, P]
        lsc = pool("lsc", 1)               # s_l columns, resident
        sml = pool("sml", 6)               # [P, 1] scratch
        opool = pool("o1", 2)
        psum = ctx.enter_context(tc.tile_pool(name="psum", bufs=3, space="PSUM"))
        psumT = ctx.enter_context(tc.tile_pool(name="psumT", bufs=2, space="PSUM"))

        ident = const_pool.tile([P, P], bf16)
        masks.make_identity(nc, ident[:])

        s_l_all = lsc.tile([P, MT], f32)
        qr_nb = [qr_pool.tile([P, KT, W], bf16, name=f"qrnb{nb}")
                 for nb in range(NB)]

        # ---- rhs stages: load+reduce, then quant+fold+xbar ----
        rraw_t, rsc_t = {}, {}

        def emit_rhs_load(j):
            raw = raws.tile([P, K], f32, name="raw")
            nc.sync.dma_start(raw[:], rhsT[j * P:(j + 1) * P, :])
            am = sml.tile([P, 1], f32, name="ram")
            nc.vector.tensor_reduce(am[:], raw[:], axis=mybir.AxisListType.X,
                                    op=Alu.max, apply_absolute_value=True)
            s_col = rsc.tile([P, 1], f32, name="rs")
            nc.vector.tensor_scalar(s_col[:], am[:], TINY, INV_QDIV,
                                    op0=Alu.max, op1=Alu.mult)
            r_col = sml.tile([P, 1], f32, name="rr")
            nc.vector.reciprocal(r_col[:], s_col[:])
            rraw_t[j] = raw
            rsc_t[j] = (s_col, r_col)

        def emit_rhs_quant(j):
            nb, jo = divmod(j, TPB)
            raw = rraw_t.pop(j)
            s_col, r_col = rsc_t.pop(j)
            for h in range(2):
                t1 = rt1.tile([P, H], f32, name="rt1")
                nc.scalar.activation(t1[:], raw[:, h * H:(h + 1) * H], Act.Copy,
                                     bias=C_MAGIC, scale=r_col[:])
                qf = rqf.tile([P, H], bf16, name="rqf")
                nc.vector.tensor_scalar(qf[:], t1[:], C_MAGIC, s_col[:],
                                        op0=Alu.subtract, op1=Alu.mult)
                nc.sync.dma_start_transpose(
                    qr_nb[nb][:, h * HT:(h + 1) * HT, jo * P:(jo + 1) * P], qf[:])

        # ---- lhs pipeline stages (full-width [P, K] tiles) ----
        lraw_t, lam_t, lqb_t, lqt_t = {}, {}, {}, {}

        def emit_lhs_load(mi):
            rs = slice(mi * P, (mi + 1) * P)
            raw = raws.tile([P, K], f32, name="raw")
            nc.sync.dma_start(raw[:], lhs[rs, :])
            am = sml.tile([P, 1], f32, name="lam")
            nc.vector.tensor_reduce(am[:], raw[:], axis=mybir.AxisListType.X,
                                    op=Alu.max, apply_absolute_value=True)
            lraw_t[mi] = raw
            lam_t[mi] = am

        def emit_lhs_quant(mi):
            raw, am = lraw_t.pop(mi), lam_t.pop(mi)
            s_col = s_l_all[:, mi:mi + 1]
            nc.vector.tensor_scalar(s_col, am[:], TINY, INV_QDIV,
                                    op0=Alu.max, op1=Alu.mult)
            r_col = sml.tile([P, 1], f32, name="lr")
            nc.vector.reciprocal(r_col[:], s_col)
            t1 = lt1.tile([P, K], f32, name="lt1")
            nc.scalar.activation(t1[:], raw[:], Act.Copy,
                                 bias=C_MAGIC, scale=r_col[:])
            qb = lqb.tile([P, K], bf16, name="lqb")
            nc.scalar.activation(qb[:], t1[:], Act.Copy, bias=-C_MAGIC)
            lqb_t[mi] = qb

        def emit_lhs_transpose(mi, xbar_kt):
            qb = lqb_t.pop(mi)
            qt = lqt.tile([P, KT, P], bf16, name="lqt")
            if xbar_kt > 0:
                nc.sync.dma_start_transpose(qt[:, 0:xbar_kt, :],
                                            qb[:, 0:xbar_kt * P])
            for g in range((KT - xbar_kt) // 8):
                pt = psumT.tile([P, 8 * P], bf16, name="pt")
                for t in range(8):
                    kt = xbar_kt + g * 8 + t
                    nc.tensor.transpose(pt[:, t * P:(t + 1) * P],
                                        qb[:, kt * P:(kt + 1) * P],
                                        ident[:])
                nc.vector.tensor_copy(qt[:, xbar_kt + g * 8:xbar_kt + (g + 1) * 8, :],
                                      pt[:])
            lqt_t[mi] = qt

        def emit_mm_group(mi, nb, last):
            qt = lqt_t[mi]
            if last:
                del lqt_t[mi]
            rs = slice(mi * P, (mi + 1) * P)
            ps = psum.tile([P, W], f32, name="ps")
            for kt in range(KT):
                nc.tensor.matmul(ps[:], qt[:, kt, :], qr_nb[nb][:, kt, :],
                                 start=(kt == 0), stop=(kt == KT - 1))
            o1 = opool.tile([P, W], f32, name="o1")
            nc.scalar.activation(o1[:], ps[:], Act.Copy, bias=0.0,
                                 scale=s_l_all[:, mi:mi + 1])
            nc.sync.dma_start(out[rs, nb * W:(nb + 1) * W], o1[:])

        # ---- emission schedule: slot-audited for the shared 4-buf raw
        # pool; rhs tiles get slot priority so tile 7 lands early, lhs
        # loads 4/5 pre-issued so the pipeline does not restart after the
        # head, steady loop runs a 3-deep lhs prefetch ----
        emit_lhs_load(0)
        emit_rhs_load(0)
        emit_rhs_load(1)
        emit_rhs_load(2)
        emit_lhs_quant(0)
        emit_lhs_transpose(0, 0)
        emit_lhs_load(1)
        emit_rhs_quant(0)
        emit_rhs_load(3)
        emit_rhs_quant(1)
        emit_rhs_load(4)
        emit_rhs_quant(2)
        emit_rhs_load(5)
        emit_lhs_quant(1)
        emit_lhs_transpose(1, 0)
        emit_lhs_load(2)
        emit_rhs_quant(3)
        emit_rhs_load(6)
        emit_rhs_quant(4)
        emit_rhs_load(7)
        emit_rhs_quant(5)
        emit_lhs_load(3)
        emit_lhs_quant(2)
        emit_lhs_transpose(2, 0)
        emit_rhs_quant(6)
        emit_rhs_quant(7)
        emit_lhs_quant(3)
        emit_lhs_transpose(3, XBAR_KT)
        emit_lhs_load(4)
        emit_lhs_load(5)
        for mi in range(HEAD_MIS):
            emit_mm_group(mi, 0, last=False)
        for mi in range(HEAD_MIS):
            emit_mm_group(mi, 1, last=True)
        emit_lhs_quant(4)
        emit_lhs_transpose(4, XBAR_KT)
        emit_lhs_load(6)

        for mi in range(HEAD_MIS, MT):
            if mi >= HEAD_MIS + 1:
                if mi + 3 < MT:
                    emit_lhs_load(mi + 3)
                if mi + 1 < MT and mi + 1 > HEAD_MIS + 1:
                    emit_lhs_quant(mi + 1)
                    emit_lhs_transpose(mi + 1, XBAR_KT)
            for nb in range(NB):
                emit_mm_group(mi, nb, last=(nb == NB - 1))
    return nc


_COMPILED_NC = None


def _get_compiled():
    global _COMPILED_NC
    if _COMPILED_NC is None:
        nc = bacc.Bacc("TRN2", target_bir_lowering=False, debug=False,
                       num_devices=N_CORES)
        build_aqt(nc, M_loc, K, N_loc)
        nc.compile()
        _COMPILED_NC = nc
    return _COMPILED_NC


def _shard(lhs, rhs):
    rhsT = np.ascontiguousarray(rhs.T)   # [N, K]; row slices stay contiguous
    in_maps = []
    for i in range(N_CORES):
        mg, ng = divmod(i, NG)
        in_maps.append({
            "lhs": np.ascontiguousarray(lhs[mg * M_loc:(mg + 1) * M_loc, :]),
            "rhsT": rhsT[ng * N_loc:(ng + 1) * N_loc, :],
        })
    return in_maps


def kernel(lhs, rhs, _trace=False, _trace_kwargs=None):
    lhs = np.asarray(lhs, np.float32)
    rhs = np.asarray(rhs, np.float32)
    nc = _get_compiled()
    res = run_bass_kernel_spmd(nc, _shard(lhs, rhs), core_ids=list(range(N_CORES)),
                               trace=_trace, **(_trace_kwargs or {}))
    out = np.empty((M, N), np.float32)
    for i in range(N_CORES):
        mg, ng = divmod(i, NG)
        out[mg * M_loc:(mg + 1) * M_loc, ng * N_loc:(ng + 1) * N_loc] = \
            res.results[i]["out"]
    kernel.last_result = res
    return out
